# revision 12
# baseline (speedup 1.0000x reference)
"""MoE BaseRouter kernel for Trainium2 (8 NeuronCores, SPMD over tokens).

Computes, for h [T=16384, D=4096] f32, W [D, E=128] f32, token_mask [T] bool:
  logits_clean = h @ W
  logits_sel   = logits_clean + gumbel          (tau=1, temps=1)
  top-8 expert mask over logits_sel (per token)
  probs        = renormalized softmax(logits_clean) over the selected experts
returning (mask_full bool, probs f32, logits_clean f32, logits_sel f32),
mirroring the reference BaseRouter.

Strategy: tokens sharded 2048/core across 8 cores; W replicated. h is
transposed on the host (hT, [D, Tshard]) so the GEMM contraction dim D sits
on SBUF partitions. Per core: 4 groups of 512 tokens; each group accumulates
32 fp32 matmuls (W chunk stationary [128d,128E], hT chunk moving [128d,512t])
into PSUM [E,512], PE-transposes 128x128 blocks to token-major, then a short
DVE/ACT stage does top-8 (single InstMax), threshold mask, exp and masked
renormalization. Gumbel noise is a fixed PRNG constant (jax threefry,
key fold_in(key(7),1)) computed on the host CPU and streamed in.
"""

import numpy as np

T, D, E, K = 16384, 4096, 128, 8
NCORES = 8
TSH = T // NCORES  # tokens per core (2048)
NG = 4  # token groups per core
GT = TSH // NG  # tokens per group (512)
NB = GT // 128  # 128-token blocks per group (4)
NK = D // 128  # contraction chunks (32)

_cache = {}


def _apply_tile_patch(tile_mod, mybir):
    """walrus CoreV3 codegen allows at most 1 sync wait per instruction; the
    TileContext tail drain carries one wait per live proc. Spill them onto
    SP nops (order only needs every wait to precede the semaphore clear)."""
    if getattr(tile_mod.TileContext, "_drain_patched", False):
        return

    def _drain_and_barrier(self, tick_clock, wait_clock):
        nc = self.nc
        drain_inst = nc.sync.drain()
        wait_clock.add_sem_waits(
            drain_inst.ins, tile_mod.ScopedClock({None: tick_clock.global_clock})
        )
        si = drain_inst.ins.sync_info
        if si is not None and si.on_wait:
            waits = list(si.on_wait)
            del si.on_wait[:]
            for w in waits:
                nop = nc.sync.nop(nofuse=True, hint="drain_wait_spill")
                nop.ins.sync_info = mybir.SyncInfo(on_update=[], on_wait=[w])
        nc.all_engine_barrier()
        assert self.sems is not None
        popped = nc._tile_sem_poison_stack.pop()
        assert popped is self._sem_poison
        nc.clear_and_free_semaphores(list(self.sems.allocated().values()))
        nc.all_engine_barrier()

    tile_mod.TileContext._drain_and_barrier = _drain_and_barrier
    tile_mod.TileContext._drain_patched = True


def _build_nc(repeat=1, fp16=True):
    """Build the per-core Bass module (same program on all 8 cores).

    repeat>1 wraps the body in an on-device For_i loop (same static program
    re-executed; used only for timing measurements).

    fp16=True streams h and W as fp16 hi/lo splits (h = h0 + h1 exactly to
    ~2^-22 rel) and accumulates three fp16 matmul terms h0W0 + h1W0 + h0W1
    in fp32 PSUM — fp16 products are exact on the PE (e10m11 upconvert,
    e10m23 product), so this matches fp32-native GEMM precision at 3/4 of
    its PE cost (fp16 streams at 1 cycle/row vs fp32's 4)."""
    from contextlib import ExitStack

    import concourse.tile as tile
    from concourse import bacc, mybir

    f32 = mybir.dt.float32
    f16 = mybir.dt.float16
    u8 = mybir.dt.uint8

    nc = bacc.Bacc(
        "TRN2", target_bir_lowering=False, debug=False, enable_asserts=False
    )
    if fp16:
        h0T = nc.dram_tensor("h0T", [D, TSH], f16, kind="ExternalInput")
        h1T = nc.dram_tensor("h1T", [D, TSH], f16, kind="ExternalInput")
        W0 = nc.dram_tensor("W0", [D, E], f16, kind="ExternalInput")
        W1 = nc.dram_tensor("W1", [D, E], f16, kind="ExternalInput")
    else:
        hT = nc.dram_tensor("hT", [D, TSH], f32, kind="ExternalInput")
        Wm = nc.dram_tensor("Wm", [D, E], f32, kind="ExternalInput")
    gum = nc.dram_tensor("gum", [TSH, E], f32, kind="ExternalInput")
    ident = nc.dram_tensor("ident", [128, 128], f32, kind="ExternalInput")
    lclean = nc.dram_tensor("lclean", [TSH, E], f32, kind="ExternalOutput")
    lsel = nc.dram_tensor("lsel", [TSH, E], f32, kind="ExternalOutput")
    probs = nc.dram_tensor("probs", [TSH, E], f32, kind="ExternalOutput")
    mask = nc.dram_tensor("mask", [TSH, E], u8, kind="ExternalOutput")

    # DRAM views: d = 128*k + p ; token = 512*g + 128*b + p
    def chunked(t, last):  # [D, X] -> [128, NK, X]
        return t[:, :].rearrange(f"(k p) {last} -> p k {last}", p=128)

    if fp16:
        h_vs = [chunked(h0T, "j"), chunked(h1T, "j")]
        W_vs = [chunked(W0, "e"), chunked(W1, "e")]
        hdt = f16
    else:
        h_vs = [chunked(hT, "j")]
        W_vs = [chunked(Wm, "e")]
        hdt = f32

    def tok_view(t):  # [TSH, E] -> [NG][128, NB, E]
        return t[:, :].rearrange("(g b p) e -> g p b e", g=NG, b=NB)

    lclean_v, lsel_v, probs_v, mask_v, gum_v = (
        tok_view(x) for x in (lclean, lsel, probs, mask, gum)
    )

    with tile.TileContext(nc) as tc, ExitStack() as ctx:
        wpool = ctx.enter_context(tc.tile_pool(name="w", bufs=1))
        cpool = ctx.enter_context(tc.tile_pool(name="const", bufs=1))
        hpool = ctx.enter_context(tc.tile_pool(name="h", bufs=16 if fp16 else 8))
        ppool = ctx.enter_context(tc.tile_pool(name="psum_mm", bufs=2, space="PSUM"))
        ptpool = ctx.enter_context(tc.tile_pool(name="psum_t", bufs=6, space="PSUM"))
        lgpool = ctx.enter_context(tc.tile_pool(name="lg", bufs=2))
        opool = ctx.enter_context(tc.tile_pool(name="outs", bufs=2))
        spool = ctx.enter_context(tc.tile_pool(name="small", bufs=16))

        def body():
            W_sbs = []
            for i, W_v in enumerate(W_vs):
                W_sb = wpool.tile([128, NK, E], hdt, tag=f"w{i}")
                nc.sync.dma_start(W_sb[:], W_v)
                W_sbs.append(W_sb)
            id_sb = cpool.tile([128, 128], f32)
            nc.sync.dma_start(id_sb[:], ident[:, :])
            _groups(W_sbs, id_sb)

        def _groups(W_sbs, id_sb):
          for g in range(NG):
            # ---- load hT group slab(s) in 4 sub-DMAs of 8 chunks each ----
            hs = []  # hs[i][q] : [128, 8, GT] chunk-slab of h-split i
            for i, h_v in enumerate(h_vs):
                hs.append([])
                for q in range(4):
                    h_sb = hpool.tile([128, 8, GT], hdt, tag="hslab")
                    nc.sync.dma_start(
                        h_sb[:], h_v[:, 8 * q : 8 * q + 8, g * GT : (g + 1) * GT]
                    )
                    hs[i].append(h_sb)

            # ---- GEMM: psum[E, GT] += sum_k sum_terms Wi[k].T @ hj[k] ----
            if fp16:
                terms = [(0, 0), (1, 0), (0, 1)]  # (h split, W split)
            else:
                terms = [(0, 0)]
            psum_g = ppool.tile([E, GT], f32, tag="psg")
            nmm = NK * len(terms)
            m = 0
            for k in range(NK):
                for hi, wi in terms:
                    nc.tensor.matmul(
                        psum_g[:],
                        W_sbs[wi][:, k, :],
                        hs[hi][k // 8][:, k % 8, :],
                        start=(m == 0),
                        stop=(m == nmm - 1),
                    )
                    m += 1

            # ---- PSUM -> SBUF, then PE-transpose to token-major blocks ----
            lg_sb = lgpool.tile([E, GT], f32, tag="lg")
            nc.scalar.copy(lg_sb[:], psum_g[:])

            gum_sb = opool.tile([128, NB, E], f32, tag="gum")
            nc.sync.dma_start(gum_sb[:], gum_v[g])
            lclean_sb = opool.tile([128, NB, E], f32, tag="lclean")
            lsel_sb = opool.tile([128, NB, E], f32, tag="lsel")
            probs_sb = opool.tile([128, NB, E], f32, tag="probs")
            mask_sb = opool.tile([128, NB, E], u8, tag="mask")

            for b in range(NB):
                lt = ptpool.tile([128, E], f32, tag="pst")  # logits [tok, E] in PSUM
                nc.tensor.transpose(lt[:], lg_sb[:, 128 * b : 128 * (b + 1)], id_sb[:])

                # logits_sel = logits + gumbel (also the lsel output tile)
                xs = lsel_sb[:, b, :]
                nc.vector.tensor_tensor(xs, lt[:], gum_sb[:, b, :], mybir.AluOpType.add)

                # top-8 values; threshold = 8th largest
                m8 = spool.tile([128, 8], f32, tag="m8")
                nc.vector.max(m8[:], xs)
                thr = m8[:, 7:8]
                maskf = spool.tile([128, E], f32, tag="maskf")
                nc.vector.tensor_scalar(maskf[:], xs, thr, None, mybir.AluOpType.is_ge)
                nc.vector.tensor_scalar(
                    mask_sb[:, b, :], xs, thr, None, mybir.AluOpType.is_ge
                )

                # renormalized softmax over selected experts
                nmax = spool.tile([128, 1], f32, tag="nmax")
                nc.vector.tensor_reduce(
                    nmax[:], lt[:], mybir.AxisListType.X, mybir.AluOpType.max,
                    negate=True,
                )
                et = spool.tile([128, E], f32, tag="et")
                nc.scalar.activation(
                    et[:], lt[:], mybir.ActivationFunctionType.Exp,
                    bias=nmax[:, 0:1], scale=1.0,
                )
                nc.scalar.copy(lclean_sb[:, b, :], lt[:])
                pb = probs_sb[:, b, :]
                nc.vector.tensor_tensor(pb, et[:], maskf[:], mybir.AluOpType.mult)
                ssum = spool.tile([128, 1], f32, tag="ssum")
                nc.vector.reduce_sum(ssum[:], pb, axis=mybir.AxisListType.X)
                rec = spool.tile([128, 1], f32, tag="rec")
                nc.vector.reciprocal(rec[:], ssum[:])
                nc.vector.tensor_scalar_mul(pb, pb, rec[:, 0:1])

            nc.scalar.dma_start(lclean_v[g], lclean_sb[:])
            nc.scalar.dma_start(lsel_v[g], lsel_sb[:])
            nc.scalar.dma_start(probs_v[g], probs_sb[:])
            nc.scalar.dma_start(mask_v[g], mask_sb[:])

        if repeat == 1:
            body()
        else:
            with tc.For_i(0, repeat, 1):
                body()

    nc.compile()
    return nc


def _gumbel_np():
    """The reference's gumbel draw — a fixed constant (jax threefry on CPU)."""
    import jax
    import jax.numpy as jnp

    cpu = jax.devices("cpu")[0]
    with jax.default_device(cpu):
        kg = jax.random.fold_in(jax.random.key(7), 1)
        u = jax.random.uniform(
            kg, (T, E), minval=1e-06, maxval=1 - 1e-06, dtype=jnp.float32
        )
        g = -jnp.log(-jnp.log(u))
        return np.asarray(jax.device_get(g), dtype=np.float32)


USE_FP16 = True


def _make_in_maps(h, W, gum):
    ident = np.eye(128, dtype=np.float32)
    in_maps = []
    if USE_FP16:
        h0 = h.astype(np.float16)
        h1 = (h - h0.astype(np.float32)).astype(np.float16)
        W0 = W.astype(np.float16)
        W1 = (W - W0.astype(np.float32)).astype(np.float16)
        for c in range(NCORES):
            sl = slice(c * TSH, (c + 1) * TSH)
            in_maps.append(
                {
                    "h0T": np.ascontiguousarray(h0[sl].T),
                    "h1T": np.ascontiguousarray(h1[sl].T),
                    "W0": W0,
                    "W1": W1,
                    "gum": np.ascontiguousarray(gum[sl]),
                    "ident": ident,
                }
            )
    else:
        Wc = np.ascontiguousarray(W, dtype=np.float32)
        for c in range(NCORES):
            sl = slice(c * TSH, (c + 1) * TSH)
            in_maps.append(
                {
                    "hT": np.ascontiguousarray(h[sl].T),
                    "Wm": Wc,
                    "gum": np.ascontiguousarray(gum[sl]),
                    "ident": ident,
                }
            )
    return in_maps


def _run_device(h, W):
    from concourse.bass_utils import run_bass_kernel_spmd

    if "nc" not in _cache:
        _cache["nc"] = _build_nc(fp16=USE_FP16)
    if "gum" not in _cache:
        _cache["gum"] = _gumbel_np()
    in_maps = _make_in_maps(h, W, _cache["gum"])
    res = run_bass_kernel_spmd(_cache["nc"], in_maps, core_ids=list(range(NCORES)))
    outs = {
        k: np.concatenate([res.results[c][k] for c in range(NCORES)], axis=0)
        for k in ("lclean", "lsel", "probs", "mask")
    }
    return outs, res


def kernel(h, W, token_mask):
    h = np.asarray(h, dtype=np.float32)
    W = np.asarray(W, dtype=np.float32)
    tm = np.asarray(token_mask).astype(bool)

    outs, _ = _run_device(h, W)
    lclean = outs["lclean"]
    lsel = outs["lsel"]
    probs = outs["probs"]
    mask = outs["mask"].astype(bool)

    if not tm.all():
        lsel[~tm] = -np.inf
        mask[~tm] = False
        probs[~tm] = 0.0

    # Exact fixup for threshold ties (rows where ">= 8th value" selected != 8):
    bad = np.flatnonzero((mask.sum(axis=1) != K) & tm)
    for r in bad:
        order = np.argsort(-lsel[r], kind="stable")[:K]
        m = np.zeros(E, dtype=bool)
        m[order] = True
        mask[r] = m
        x = lclean[r].astype(np.float32)
        e = np.exp(x - x.max(), dtype=np.float32)
        p = (e / e.sum()).astype(np.float32)
        mp = np.where(m, p, np.float32(0.0))
        denom = np.maximum(mp.sum(), np.float32(1e-09))
        probs[r] = mp / denom

    return mask, probs, lclean, lsel


# revision 27
# speedup vs baseline: 1.0831x; 1.0831x over previous
"""MoE BaseRouter kernel for Trainium2 (8 NeuronCores, SPMD over tokens).

Computes, for h [T=16384, D=4096] f32, W [D, E=128] f32, token_mask [T] bool:
  logits_clean = h @ W
  logits_sel   = logits_clean + gumbel          (tau=1, temps=1)
  top-8 expert mask over logits_sel (per token)
  probs        = renormalized softmax(logits_clean) over the selected experts
returning (mask_full bool, probs f32, logits_clean f32, logits_sel f32),
mirroring the reference BaseRouter.

Strategy: tokens sharded 2048/core across 8 cores; W replicated. h is
transposed on the host (hT, [D, Tshard]) so the GEMM contraction dim D sits
on SBUF partitions. Per core: 4 groups of 512 tokens; each group accumulates
32 fp32 matmuls (W chunk stationary [128d,128E], hT chunk moving [128d,512t])
into PSUM [E,512], PE-transposes 128x128 blocks to token-major, then a short
DVE/ACT stage does top-8 (single InstMax), threshold mask, exp and masked
renormalization. Gumbel noise is a fixed PRNG constant (jax threefry,
key fold_in(key(7),1)) computed on the host CPU and streamed in.
"""

import numpy as np

T, D, E, K = 16384, 4096, 128, 8
NCORES = 8
TSH = T // NCORES  # tokens per core (2048)
NG = 4  # token groups per core
GT = TSH // NG  # tokens per group (512)
NB = GT // 128  # 128-token blocks per group (4)
NK = D // 128  # contraction chunks (32)

_cache = {}


def _build_nc(repeat=1, fp16=True, mode="full", ng=NG, row_split=False):
    """Build the per-core Bass module (same program on all 8 cores).

    repeat>1 wraps the body in an on-device For_i loop (same static program
    re-executed; used only for timing measurements).

    fp16=True streams h and W as fp16 hi/lo splits (h = h0 + h1 exactly to
    ~2^-22 rel) and accumulates three fp16 matmul terms h0W0 + h1W0 + h0W1
    in fp32 PSUM — fp16 products are exact on the PE (e10m11 upconvert,
    e10m23 product), so this matches fp32-native GEMM precision at 3/4 of
    its PE cost (fp16 streams at 1 cycle/row vs fp32's 4)."""
    from contextlib import ExitStack

    import concourse.tile as tile
    from concourse import bacc, mybir

    f32 = mybir.dt.float32
    f16 = mybir.dt.float16
    u8 = mybir.dt.uint8

    nc = bacc.Bacc(
        "TRN2", target_bir_lowering=False, debug=False, enable_asserts=False
    )
    if fp16:
        # packed host layouts: h*[g, p, k, j] (d = 128k+p, tok = gt*g + j),
        # W*[p, k, e], gum[g, p, b, e] — every partition's slab bytes are
        # contiguous in DRAM so DMA descriptors are a few KB each.
        h0T = nc.dram_tensor("h0T", [NG, 128, NK, GT], f16, kind="ExternalInput")
        h1T = nc.dram_tensor("h1T", [NG, 128, NK, GT], f16, kind="ExternalInput")
        W0 = nc.dram_tensor("W0", [128, NK, E], f16, kind="ExternalInput")
        W1 = nc.dram_tensor("W1", [128, NK, E], f16, kind="ExternalInput")
    else:
        hT = nc.dram_tensor("hT", [D, TSH], f32, kind="ExternalInput")
        Wm = nc.dram_tensor("Wm", [D, E], f32, kind="ExternalInput")
    gum = nc.dram_tensor("gum", [NG, 128, NB, E], f32, kind="ExternalInput")
    ident = nc.dram_tensor("ident", [128, 128], f32, kind="ExternalInput")
    lclean = nc.dram_tensor("lclean", [TSH, E], f32, kind="ExternalOutput")
    lsel = nc.dram_tensor("lsel", [TSH, E], f32, kind="ExternalOutput")
    probs = nc.dram_tensor("probs", [TSH, E], f32, kind="ExternalOutput")
    mask = nc.dram_tensor("mask", [TSH, E], u8, kind="ExternalOutput")

    # DRAM views: d = 128*k + p ; token = 512*g + 128*b + p
    def chunked(t, last):  # [D, X] -> [128, NK, X]
        return t[:, :].rearrange(f"(k p) {last} -> p k {last}", p=128)

    if fp16:
        assert ng == NG, "packed layout bakes in NG groups"
        h_vs = [h0T[:, :, :, :], h1T[:, :, :, :]]  # [NG, 128, NK, GT]
        W_vs = [W0[:, :, :], W1[:, :, :]]  # [128, NK, E]
        hdt = f16
    else:
        h_vs = [chunked(hT, "j")]
        W_vs = [chunked(Wm, "e")]
        hdt = f32

    nb = TSH // ng // 128  # blocks per group
    gt = TSH // ng  # tokens per group

    def tok_view(t):  # [TSH, E] -> [ng][128, nb, E]
        return t[:, :].rearrange("(g b p) e -> g p b e", g=ng, b=nb)

    lclean_v, lsel_v, probs_v, mask_v = (
        tok_view(x) for x in (lclean, lsel, probs, mask)
    )
    gum_v = gum[:, :, :, :]

    with tile.TileContext(nc) as tc, ExitStack() as ctx:
        wpool = ctx.enter_context(tc.tile_pool(name="w", bufs=1))
        cpool = ctx.enter_context(tc.tile_pool(name="const", bufs=1))
        hpool = ctx.enter_context(tc.tile_pool(name="h", bufs=16 if fp16 else 8))
        ppool = ctx.enter_context(tc.tile_pool(name="psum_mm", bufs=2, space="PSUM"))
        ptpool = ctx.enter_context(tc.tile_pool(name="psum_t", bufs=6, space="PSUM"))
        lgpool = ctx.enter_context(tc.tile_pool(name="lg", bufs=2))
        opool = ctx.enter_context(tc.tile_pool(name="outs", bufs=2))
        spool = ctx.enter_context(tc.tile_pool(name="small", bufs=16))

        def body():
            W_sbs = []
            for i, W_v in enumerate(W_vs):
                W_sb = wpool.tile([128, NK, E], hdt, tag=f"w{i}")
                nc.scalar.dma_start(W_sb[:], W_v)
                W_sbs.append(W_sb)
            id_sb = cpool.tile([128, 128], f32)
            nc.scalar.dma_start(id_sb[:], ident[:, :])
            _groups(W_sbs, id_sb)

        def _groups(W_sbs, id_sb):
          nq = max(1, gt * 8 // GT)  # sub-DMA count scales with group width
          for g in range(ng):
            # ---- load hT group slab(s) in 4 sub-DMAs of 8 chunks each ----
            kq = NK // nq  # chunks per sub-DMA
            hs = [[] for _ in h_vs]  # hs[i][q] : [128, kq, gt] slab of h-split i
            for q in range(nq):
                for i, h_v in enumerate(h_vs):
                    h_sb = hpool.tile([128, kq, gt], hdt, tag="hslab")
                    if fp16:
                        src_ap = h_v[g, :, kq * q : kq * (q + 1), :]
                    else:
                        src_ap = h_v[:, kq * q : kq * (q + 1), g * gt : (g + 1) * gt]
                    nc.sync.dma_start(h_sb[:], src_ap)
                    hs[i].append(h_sb)

            if mode == "dma":
                # diagnostic: just touch the slabs with one tiny reduce each
                zt = spool.tile([128, 1], f32, tag="zt")
                for i in range(len(h_vs)):
                    for q in range(nq):
                        nc.vector.tensor_reduce(
                            zt[:], hs[i][q][:, 0, 0:2], mybir.AxisListType.X,
                            mybir.AluOpType.max,
                        )
                continue

            # ---- GEMM: psum[E, GT] += sum_k sum_terms Wi[k].T @ hj[k] ----
            if fp16:
                terms = [(0, 0), (1, 0), (0, 1)]  # (h split, W split)
                if mode in ("gemm1", "gemmA"):
                    terms = [(0, 0)]
                elif mode == "gemm2":
                    terms = [(0, 0), (1, 0)]
            else:
                terms = [(0, 0)]
            psum_g = ppool.tile([E, gt], f32, tag="psg")
            nmm = NK * len(terms)
            m = 0
            # q-major, term-minor: the first MMs of a group need only the
            # first h0 slab + W0 (2MB) instead of every first slab + both Ws.
            for q in range(nq):
                for hi, wi in terms:
                    for kk in range(kq):
                        k = q * kq + kk
                        w_ap = W_sbs[wi][:, k, :]
                        h_ap = hs[hi][q][:, kk, :]
                        if row_split:
                            nc.tensor.matmul(
                                psum_g[:], w_ap[0:64, :], h_ap[0:64, :],
                                start=(m == 0), stop=False,
                            )
                            nc.tensor.matmul(
                                psum_g[:], w_ap[64:128, :], h_ap[64:128, :],
                                start=False, stop=(m == nmm - 1),
                            )
                        else:
                            nc.tensor.matmul(
                                psum_g[:], w_ap, h_ap,
                                start=(m == 0), stop=(m == nmm - 1),
                            )
                        m += 1

            if mode in ("gemm", "gemmA"):
                lg_sb = lgpool.tile([E, gt], f32, tag="lg")
                nc.scalar.copy(lg_sb[:], psum_g[:])
                nc.scalar.dma_start(
                    lclean_v[g], lg_sb[:].rearrange("p (b e) -> p b e", e=E)
                )
                continue

            # ---- PSUM -> SBUF, then PE-transpose to token-major blocks ----
            lg_sb = lgpool.tile([E, gt], f32, tag="lg")
            nc.scalar.copy(lg_sb[:], psum_g[:])

            gum_sb = opool.tile([128, nb, E], f32, tag="gum")
            nc.sync.dma_start(gum_sb[:], gum_v[g])
            lclean_sb = opool.tile([128, nb, E], f32, tag="lclean")
            lsel_sb = opool.tile([128, nb, E], f32, tag="lsel")
            probs_sb = opool.tile([128, nb, E], f32, tag="probs")
            mask_sb = opool.tile([128, nb, E], u8, tag="mask")

            for b in range(nb):
                lt = ptpool.tile([128, E], f32, tag="pst")  # logits [tok, E] in PSUM
                nc.tensor.transpose(lt[:], lg_sb[:, 128 * b : 128 * (b + 1)], id_sb[:])

                # logits_sel = logits + gumbel (also the lsel output tile)
                xs = lsel_sb[:, b, :]
                nc.vector.tensor_tensor(xs, lt[:], gum_sb[:, b, :], mybir.AluOpType.add)

                # top-8 values; threshold = 8th largest
                m8 = spool.tile([128, 8], f32, tag="m8")
                nc.vector.max(m8[:], xs)
                thr = m8[:, 7:8]
                maskf = spool.tile([128, E], f32, tag="maskf")
                nc.vector.tensor_scalar(maskf[:], xs, thr, None, mybir.AluOpType.is_ge)
                nc.vector.tensor_scalar(
                    mask_sb[:, b, :], xs, thr, None, mybir.AluOpType.is_ge
                )

                # renormalized softmax over selected experts
                nmax = spool.tile([128, 1], f32, tag="nmax")
                nc.vector.tensor_reduce(
                    nmax[:], lt[:], mybir.AxisListType.X, mybir.AluOpType.max,
                    negate=True,
                )
                et = spool.tile([128, E], f32, tag="et")
                nc.scalar.activation(
                    et[:], lt[:], mybir.ActivationFunctionType.Exp,
                    bias=nmax[:, 0:1], scale=1.0,
                )
                nc.scalar.copy(lclean_sb[:, b, :], lt[:])
                pb = probs_sb[:, b, :]
                nc.vector.tensor_tensor(pb, et[:], maskf[:], mybir.AluOpType.mult)
                ssum = spool.tile([128, 1], f32, tag="ssum")
                nc.vector.reduce_sum(ssum[:], pb, axis=mybir.AxisListType.X)
                rec = spool.tile([128, 1], f32, tag="rec")
                nc.vector.reciprocal(rec[:], ssum[:])
                nc.vector.tensor_scalar_mul(pb, pb, rec[:, 0:1])

            nc.scalar.dma_start(lclean_v[g], lclean_sb[:])
            nc.scalar.dma_start(lsel_v[g], lsel_sb[:])
            nc.scalar.dma_start(probs_v[g], probs_sb[:])
            nc.scalar.dma_start(mask_v[g], mask_sb[:])

        if repeat == 1:
            body()
        else:
            with tc.For_i(0, repeat, 1):
                body()

    nc.compile()
    return nc


def _gumbel_np():
    """The reference's gumbel draw — a fixed constant (jax threefry on CPU)."""
    import jax
    import jax.numpy as jnp

    cpu = jax.devices("cpu")[0]
    with jax.default_device(cpu):
        kg = jax.random.fold_in(jax.random.key(7), 1)
        u = jax.random.uniform(
            kg, (T, E), minval=1e-06, maxval=1 - 1e-06, dtype=jnp.float32
        )
        g = -jnp.log(-jnp.log(u))
        return np.asarray(jax.device_get(g), dtype=np.float32)


USE_FP16 = True


def _make_in_maps(h, W, gum):
    ident = np.eye(128, dtype=np.float32)
    in_maps = []
    if USE_FP16:
        h0 = h.astype(np.float16)
        h1 = (h - h0.astype(np.float32)).astype(np.float16)

        def pack_h(x):  # [TSH, D] -> [NG, 128(p), NK, GT] (d=128k+p, t=g*GT+j)
            return np.ascontiguousarray(
                x.reshape(NG, GT, NK, 128).transpose(0, 3, 2, 1)
            )

        def pack_w(w):  # [D, E] -> [128(p), NK, E]
            return np.ascontiguousarray(w.reshape(NK, 128, E).transpose(1, 0, 2))

        def pack_g(x):  # [TSH, E] -> [NG, 128(p), NB, E]
            return np.ascontiguousarray(
                x.reshape(NG, NB, 128, E).transpose(0, 2, 1, 3)
            )

        W0p = pack_w(W.astype(np.float16))
        W1p = pack_w((W - W.astype(np.float16).astype(np.float32)).astype(np.float16))

        def core_map(c):
            sl = slice(c * TSH, (c + 1) * TSH)
            return {
                "h0T": pack_h(h0[sl]),
                "h1T": pack_h(h1[sl]),
                "W0": W0p,
                "W1": W1p,
                "gum": pack_g(gum[sl]),
                "ident": ident,
            }

        from concurrent.futures import ThreadPoolExecutor

        with ThreadPoolExecutor(max_workers=NCORES) as ex:
            in_maps = list(ex.map(core_map, range(NCORES)))
    else:
        Wc = np.ascontiguousarray(W, dtype=np.float32)
        for c in range(NCORES):
            sl = slice(c * TSH, (c + 1) * TSH)
            in_maps.append(
                {
                    "hT": np.ascontiguousarray(h[sl].T),
                    "Wm": Wc,
                    "gum": np.ascontiguousarray(gum[sl]),
                    "ident": ident,
                }
            )
    return in_maps


def _run_device(h, W):
    from concourse.bass_utils import run_bass_kernel_spmd

    if "nc" not in _cache:
        _cache["nc"] = _build_nc(fp16=USE_FP16)
    if "gum" not in _cache:
        _cache["gum"] = _gumbel_np()
    in_maps = _make_in_maps(h, W, _cache["gum"])
    res = run_bass_kernel_spmd(_cache["nc"], in_maps, core_ids=list(range(NCORES)))
    outs = {
        k: np.concatenate([res.results[c][k] for c in range(NCORES)], axis=0)
        for k in ("lclean", "lsel", "probs", "mask")
    }
    return outs, res


def kernel(h, W, token_mask):
    h = np.asarray(h, dtype=np.float32)
    W = np.asarray(W, dtype=np.float32)
    tm = np.asarray(token_mask).astype(bool)

    outs, _ = _run_device(h, W)
    lclean = outs["lclean"]
    lsel = outs["lsel"]
    probs = outs["probs"]
    mask = outs["mask"].astype(bool)

    if not tm.all():
        lsel[~tm] = -np.inf
        mask[~tm] = False
        probs[~tm] = 0.0

    # Exact fixup for threshold ties (rows where ">= 8th value" selected != 8):
    bad = np.flatnonzero((mask.sum(axis=1) != K) & tm)
    for r in bad:
        order = np.argsort(-lsel[r], kind="stable")[:K]
        m = np.zeros(E, dtype=bool)
        m[order] = True
        mask[r] = m
        x = lclean[r].astype(np.float32)
        e = np.exp(x - x.max(), dtype=np.float32)
        p = (e / e.sum()).astype(np.float32)
        mp = np.where(m, p, np.float32(0.0))
        denom = np.maximum(mp.sum(), np.float32(1e-09))
        probs[r] = mp / denom

    return mask, probs, lclean, lsel
